# revision 71
# baseline (speedup 1.0000x reference)
"""Trainium2 Bass kernel for nn_AttnLayerV3 (differential attention layer).

Strategy: tensor-parallel over heads — 16 heads across 8 NeuronCores
(2 heads/core, each core needs exactly one of the 4 GQA KV heads).
Each core receives x^T plus its weight slices, computes its heads'
attention + per-head GroupNorm + its slice of the output projection,
and returns a partial [T, D] product; the host sums the 8 partials.

All tensors stream as bf16 (halves DMA + SBUF, 2x DVE rate; PE rate is
1 col/cycle for both bf16 and f32r, accumulation stays fp32 in PSUM).
GroupNorm weight and the (1 - lambda_init) factor are folded into Wo on
the host; the gn_b term is a constant row vector added on the host.
Phase 1 walks t-strips forward so phase 2 (ascending t-strips) never
waits on RoPE; PSUM pools are sized for full cross-strip double
buffering to keep the PE p-state warm.
"""

import math
from contextlib import ExitStack

import numpy as np
import ml_dtypes

import concourse.bacc as bacc
import concourse.tile as tile
from concourse import mybir
from concourse.bass_utils import run_bass_kernel_spmd

f32 = mybir.dt.float32
f32r = mybir.dt.float32r
bf16 = mybir.dt.bfloat16
FT = mybir.ActivationFunctionType
ALU = mybir.AluOpType

# problem shape (hardcoded per contract)
B, T, D, H, KV = 1, 2048, 2048, 16, 4
DH = D // H                    # 128
REP = H // KV                  # 4
NCORES = 8
HPC = H // NCORES              # heads per core = 2
CH = HPC * 2 * DH              # 512 output channels per core
LAMBDA_INIT = 0.8 - 0.6 * math.exp(-0.3 * 0)  # 0.2
ROPE_BASE = 10000.0
EPS = 1e-5

SWAP_MASK = [i ^ 1 for i in range(32)]


# ----------------------------------------------------------------------------
# device program
# ----------------------------------------------------------------------------

def _declare_params(nc):
    p = {}
    p["xT"] = nc.dram_tensor("xT", [D, T], bf16, kind="ExternalInput").ap()
    p["wq"] = nc.dram_tensor("wq", [D, CH], bf16, kind="ExternalInput").ap()
    p["wk"] = nc.dram_tensor("wk", [D, 2 * DH], bf16, kind="ExternalInput").ap()
    p["wv"] = nc.dram_tensor("wv", [D, 2 * DH], bf16, kind="ExternalInput").ap()
    p["wo"] = nc.dram_tensor("wo", [CH, D], bf16, kind="ExternalInput").ap()
    p["ropec"] = nc.dram_tensor("ropec", [128, T], bf16, kind="ExternalInput").ap()
    p["ropes"] = nc.dram_tensor("ropes", [128, T], bf16, kind="ExternalInput").ap()
    p["tri"] = nc.dram_tensor("tri", [128, 128], bf16, kind="ExternalInput").ap()
    p["ones"] = nc.dram_tensor("ones", [128, 128], f32r, kind="ExternalInput").ap()
    p["ident"] = nc.dram_tensor("ident", [128, 128], bf16, kind="ExternalInput").ap()
    p["y"] = nc.dram_tensor("y", [T, D], bf16, kind="ExternalOutput").ap()
    return p


def _build(ctx, tc, p, lam):
    nc = tc.nc
    ND = D // 128          # contraction chunks
    NT = T // 128          # t subtiles
    S1 = 256               # phase-1 t-strip width
    NS1 = T // S1
    CH = HPC * 2 * DH      # output channels this core
    NCH = CH // 128
    NQ = 2 * HPC           # q rows (h-major, [q1,q2] minor)
    scale = 1.0 / math.sqrt(DH)

    # ---------------- constant tiles (resident) ----------------
    consts = ctx.enter_context(tc.tile_pool(name="consts", bufs=1))
    ropec_sb = consts.tile([128, T], bf16, tag="ropec")
    ropes_sb = consts.tile([128, T], bf16, tag="ropes")
    tri_sb = consts.tile([128, 128], bf16, tag="tri")
    ones_sb = consts.tile([128, 128], f32r, tag="ones")
    ident_sb = consts.tile([128, 128], bf16, tag="ident")
    eps_sb = consts.tile([128, 1], f32, tag="eps")
    nc.vector.memset(eps_sb[:], EPS)

    # outT outlives the q/k/v tensors (read by phase 3), so its pool sits
    # below them on the pool stack.
    persist = ctx.enter_context(tc.tile_pool(name="persist", bufs=1))
    outT_sb = persist.tile([128, NCH, T], bf16, tag="outT")      # [ch, chunk, t]

    acts = ctx.enter_context(tc.tile_pool(name="acts", bufs=1))
    qT_sb = acts.tile([128, NQ, T], bf16, tag="qT")              # [dh, (h,b), t]
    kT_sb = acts.tile([128, 2, T], bf16, tag="kT")               # [dh, b, t]
    vaug_sb = acts.tile([128, NT, 2 * DH + 2], bf16, tag="vaug")  # [s, sblock, ch|1]
    nc.vector.memset(vaug_sb[:, :, 2 * DH:2 * DH + 2], 1.0)

    # phase-2/3 SBUF pools created before phase 1's transient pools so their
    # zones never overlap a released zone (no cross-phase barrier waits).
    wop = ctx.enter_context(tc.tile_pool(name="wop", bufs=1))
    wo_sb = wop.tile([128, NCH, D], bf16, tag="wo")
    wor = p["wo"].rearrange("(n p) m -> p n m", p=128)
    epool = ctx.enter_context(tc.tile_pool(name="exp", bufs=24))
    o1pool = ctx.enter_context(tc.tile_pool(name="o1sp", bufs=10))
    cpool = ctx.enter_context(tc.tile_pool(name="comb", bufs=4))
    spool = ctx.enter_context(tc.tile_pool(name="small", bufs=8))
    gpool = ctx.enter_context(tc.tile_pool(name="gn", bufs=2))
    ypool = ctx.enter_context(tc.tile_pool(name="yst", bufs=4))

    # one PSUM pool for all phases: phase-1 accumulators borrow the
    # phase-2/3 tags (even strips: op x4, odd strips: sc x3 + pt x1) so no
    # pool-release barrier separates the phases.
    pp = ctx.enter_context(tc.tile_pool(name="pp", bufs=1, space="PSUM"))

    # ================= phase 1: projections + RoPE =================
    def rope_pass(rpool, dest, ps, t0, w, ceng):
        raw = rpool.tile([128, w], bf16, tag="raw", name="raw")
        if ceng == 0:
            nc.scalar.copy(raw[:], ps)
        else:
            nc.vector.tensor_copy(raw[:], ps)
        swp = rpool.tile([128, w], bf16, tag="swp", name="swp")
        nc.vector.stream_shuffle(swp[:], raw[:], SWAP_MASK)
        nc.gpsimd.tensor_tensor(dest, raw[:], ropec_sb[:, t0:t0 + w], ALU.mult)
        nc.vector.tensor_tensor(swp[:], swp[:], ropes_sb[:, t0:t0 + w], ALU.mult)
        nc.vector.tensor_tensor(dest, dest, swp[:], ALU.add)

    with ExitStack() as ph1:
        wts = ph1.enter_context(tc.tile_pool(name="wts", bufs=1))
        wq_sb = wts.tile([128, ND, CH], bf16, tag="wq")
        wk_sb = wts.tile([128, ND, 2 * DH], bf16, tag="wk")
        wv_sb = wts.tile([128, ND, 2 * DH], bf16, tag="wv")
        wqr = p["wq"].rearrange("(n p) m -> p n m", p=128)
        wkr = p["wk"].rearrange("(n p) m -> p n m", p=128)
        wvr = p["wv"].rearrange("(n p) m -> p n m", p=128)
        for dg in range(ND // 4):
            if dg == 0:
                # d0 alone first so the first matmuls unblock early, then
                # the rest of the group as one descriptor
                nc.scalar.dma_start(wq_sb[:, 0:1, :], wqr[:, 0:1, :])
                nc.scalar.dma_start(wq_sb[:, 1:4, :], wqr[:, 1:4, :])
            else:
                dsl = slice(dg * 4, (dg + 1) * 4)
                nc.scalar.dma_start(wq_sb[:, dsl, :], wqr[:, dsl, :])
            if dg == ND // 4 - 1:
                # first halves only: strip 0's rope needs cols 0:256; the
                # rest loads once the startup bandwidth crunch is over
                nc.scalar.dma_start(ropec_sb[:, 0:1024], p["ropec"][:, 0:1024])
                nc.scalar.dma_start(ropes_sb[:, 0:1024], p["ropes"][:, 0:1024])


        xpool = ph1.enter_context(tc.tile_pool(name="xt", bufs=8))
        rpool = ph1.enter_context(tc.tile_pool(name="rope", bufs=3))

        DB = 4  # d-chunks per x DMA
        xTr = p["xT"].rearrange("(n p) m -> p n m", p=128)
        # strip list: 7 x 256 plus a split 128+128 tail so the final
        # rope-drain chain (which gates phase 2) is half as long
        strips = [(i * S1, S1) for i in range(NS1 - 2)]
        strips += [(T - k * 128, 128) for k in (4, 3, 2, 1)]
        for s, (t0, w) in enumerate(strips):
            # acc0 = q j0|j1, acc1 = q j2|j3, acc2 = k b0|b1, acc3 = v s0|s1
            if s % 2 == 0:
                accs = [pp.tile([128, 2 * w], f32, tag="op", bufs=4,
                                name=f"acc{_i}") for _i in range(4)]
            else:
                accs = [pp.tile([128, 2 * w], f32, tag="sc", bufs=4,
                                name=f"acc{_i}") for _i in range(3)]
                accs.append(pp.tile([128, 2 * w], f32, tag="op", bufs=4,
                                    name="acc3"))
            for dg in range(ND // DB):
                xt = xpool.tile([128, DB, S1], bf16, tag="xt",
                                padded_shape=[128, DB, S1])
                nc.sync.dma_start(xt[:, :, 0:w],
                                  xTr[:, dg * DB:(dg + 1) * DB, t0:t0 + w])
                if s == 0 and dg < ND // DB:
                    dsl = slice(dg * DB, (dg + 1) * DB)
                    nc.sync.dma_start(wk_sb[:, dsl, :], wkr[:, dsl, :])
                    nc.sync.dma_start(wv_sb[:, dsl, :], wvr[:, dsl, :])
                if s == 1 and dg == 0:
                    nc.scalar.dma_start(tri_sb[:], p["tri"][:])
                    nc.scalar.dma_start(ones_sb[:], p["ones"][:])
                    nc.scalar.dma_start(ident_sb[:], p["ident"][:])
                if s == 2 and dg == 0:
                    nc.scalar.dma_start(ropec_sb[:, 1024:], p["ropec"][:, 1024:])
                    nc.scalar.dma_start(ropes_sb[:, 1024:], p["ropes"][:, 1024:])
                if s == 4 and dg == 0:
                    for c in range(NCH):
                        nc.scalar.dma_start(wo_sb[:, c, :], wor[:, c, :])
                for dq in range(DB):
                    d = dg * DB + dq
                    # Two accumulation groups share each PSUM bank: start=True
                    # zeroes the whole 2KB zero-region, so only the lo half
                    # starts the bank and only the hi half stops it; the hi
                    # group's first matmul accumulates onto the zeroed region.
                    for j in range(NQ):
                        nc.tensor.matmul(
                            accs[j // 2][:, (j % 2) * w:(j % 2 + 1) * w],
                            wq_sb[:, d, j * 128:(j + 1) * 128],
                            xt[:, dq, 0:w],
                            start=(d == 0 and j % 2 == 0),
                            stop=(d == ND - 1 and j % 2 == 1))
                    for j in range(2):
                        nc.tensor.matmul(
                            accs[2][:, j * w:(j + 1) * w],
                            wk_sb[:, d, j * 128:(j + 1) * 128],
                            xt[:, dq, 0:w],
                            start=(d == 0 and j == 0),
                            stop=(d == ND - 1 and j == 1))
                    for js in range(w // 128):
                        nc.tensor.matmul(
                            accs[3][:, js * 2 * DH:js * 2 * DH + 2 * DH],
                            xt[:, dq, js * 128:(js + 1) * 128],
                            wv_sb[:, d, :],
                            start=(d == 0 and js == 0),
                            stop=(d == ND - 1 and js == w // 128 - 1))
            last = (s >= len(strips) - 2)
            for j in range(2):
                rope_pass(rpool, kT_sb[:, j, t0:t0 + w],
                          accs[2][:, j * w:(j + 1) * w], t0, w,
                          1 if last else j % 2)
            for j in range(NQ):
                rope_pass(rpool, qT_sb[:, j, t0:t0 + w],
                          accs[j // 2][:, (j % 2) * w:(j % 2 + 1) * w], t0, w,
                          1 if last else j % 2)
            for js in range(w // 128):
                nc.scalar.copy(vaug_sb[:, t0 // 128 + js, 0:2 * DH],
                               accs[3][:, js * 2 * DH:js * 2 * DH + 2 * DH])

    # ================= phases 2+3 share one PSUM pool =================

    if True:
        SW = 512                     # phase-2 strip width
        NSW = T // SW
        nkk = SW // 128              # 4 t-subtiles per strip
        statmap = {}
        for h in range(HPC):
            stats = gpool.tile([128, NT, 6], f32, tag="stats", name=f"stats{h}")
            ptile = {}
            for jt in range(NSW):
                t0 = jt * SW
                nsb = (t0 + SW) // 128
                o1sp = {}                     # kk -> o1|r1 spilled to SBUF
                op = {}
                for b in (0, 1):
                    ets = {}
                    # wave A: scores + exp + PV chains for kk in 0..nkk/2
                    for kk in range(nkk // 2):
                        op[kk] = pp.tile([128, 512], f32, tag="op", bufs=4,
                                         name=f"op{b}_{kk}")
                    for sb in range(nsb):
                        sc = pp.tile([128, SW], f32, tag="sc", bufs=4)
                        et = epool.tile([128, SW], bf16, tag="et")
                        if sb * 128 >= t0:   # diagonal sub-block: mask s > t
                            kd = sb - t0 // 128
                            c0 = kd * 128
                            nc.tensor.matmul(sc[:, c0:],
                                             kT_sb[:, b, sb * 128:(sb + 1) * 128],
                                             qT_sb[:, h * 2 + b, t0 + c0:t0 + SW],
                                             start=True, stop=True)
                            nc.scalar.activation(et[:, kd * 128:], sc[:, kd * 128:],
                                                 FT.Exp, scale=scale)
                            nc.gpsimd.tensor_tensor(
                                et[:, kd * 128:(kd + 1) * 128],
                                et[:, kd * 128:(kd + 1) * 128], tri_sb[:], ALU.mult)
                        else:
                            nc.tensor.matmul(sc[:],
                                             kT_sb[:, b, sb * 128:(sb + 1) * 128],
                                             qT_sb[:, h * 2 + b, t0:t0 + SW],
                                             start=True, stop=True)
                            nc.scalar.activation(et[:], sc[:], FT.Exp, scale=scale)
                        ets[sb] = et
                        for kk in range(nkk // 2):
                            tb = t0 // 128 + kk
                            if sb > tb:
                                continue
                            nc.tensor.matmul(op[kk][:, 0:2 * DH + 2],
                                             et[:, kk * 128:(kk + 1) * 128],
                                             vaug_sb[:, sb, :],
                                             start=(sb == 0), stop=(sb == tb))
                    if b == 0:
                        for kk in range(nkk // 2):
                            o1sp[kk] = o1pool.tile([128, 2 * DH + 2], f32,
                                                   tag="o1sp", name=f"o1sp{kk}")
                            nc.vector.tensor_copy(o1sp[kk][:],
                                                  op[kk][:, 0:2 * DH + 2])
                    # wave B: pure-PE PV chains for kk in nkk/2..nkk (re-reads et)
                    for kk in range(nkk // 2, nkk):
                        op[kk] = pp.tile([128, 512], f32, tag="op", bufs=4,
                                         name=f"op{b}_{kk}")
                    for sb in range(nsb):
                        for kk in range(nkk // 2, nkk):
                            tb = t0 // 128 + kk
                            if sb > tb:
                                continue
                            nc.tensor.matmul(op[kk][:, 0:2 * DH + 2],
                                             ets[sb][:, kk * 128:(kk + 1) * 128],
                                             vaug_sb[:, sb, :],
                                             start=(sb == 0), stop=(sb == tb))
                    if b == 0:
                        for kk in range(nkk // 2, nkk):
                            o1sp[kk] = o1pool.tile([128, 2 * DH + 2], f32,
                                                   tag="o1sp", name=f"o1sp{kk}")
                            nc.vector.tensor_copy(o1sp[kk][:],
                                                  op[kk][:, 0:2 * DH + 2])
                def combine(kk):
                    tb = t0 // 128 + kk
                    inv1 = spool.tile([128, 1], f32, tag="inv1")
                    nc.vector.reciprocal(inv1[:], o1sp[kk][:, 2 * DH:2 * DH + 1])
                    inv2 = spool.tile([128, 1], f32, tag="inv2")
                    nc.vector.reciprocal(inv2[:], op[kk][:, 2 * DH:2 * DH + 1])
                    nlinv2 = spool.tile([128, 1], f32, tag="nlinv2")
                    nc.vector.tensor_scalar_mul(nlinv2[:], inv2[:], -lam)
                    o1s = cpool.tile([128, 2 * DH], f32, tag="o1s")
                    nc.vector.tensor_scalar_mul(o1s[:], o1sp[kk][:, 0:2 * DH],
                                                inv1[:])
                    ot = cpool.tile([128, 2 * DH], bf16, tag="ot")
                    nc.vector.scalar_tensor_tensor(ot[:], op[kk][:, 0:2 * DH],
                                                   nlinv2[:], o1s[:],
                                                   ALU.mult, ALU.add)
                    nc.vector.bn_stats(stats[:, tb, :], ot[:])
                    if tb % 2 == 0:
                        ptile[0] = pp.tile([128, 512], bf16, tag="op", bufs=4,
                                           name="pt")
                    for c in range(2):
                        nc.tensor.matmul(
                            ptile[0][:, (c * 2 + tb % 2) * 128:
                                     (c * 2 + tb % 2 + 1) * 128],
                            ot[:, c * 128:(c + 1) * 128], ident_sb[:],
                            is_transpose=True,
                            start=(tb % 2 == 0 and c == 0),
                            stop=(tb % 2 == 1 and c == 1))
                    if tb % 2 == 1:
                        for c in range(2):
                            dst = outT_sb[:, h * 2 + c,
                                          (tb - 1) * 128:(tb + 1) * 128]
                            nc.vector.tensor_copy(
                                dst, ptile[0][:, c * 256:(c + 1) * 256])

                for kk in range(nkk):
                    combine(kk)

            statmap[h] = stats


        # ---- GroupNorm for both heads at phase-2 end: a single act-table
        # switch (Sqrt), and the scale pass runs q4-outer so phase 3's first
        # t-blocks unblock after one chunk per channel tile.
        rstds, mrss = {}, {}
        for h in range(HPC):
            stats = statmap[h]
            mv = gpool.tile([128, 2], f32, tag="mv", name="mv")
            nc.vector.bn_aggr(mv[:], stats[:])
            msq = gpool.tile([128, 1], f32, tag="msq", name="msq")
            nc.vector.tensor_tensor(msq[:], mv[:, 0:1], mv[:, 0:1], ALU.mult)
            tmp2 = gpool.tile([128, 2], f32r, tag="tmp2", name="tmp2")
            nc.vector.tensor_copy(tmp2[:, 0:1], mv[:, 0:1])
            nc.vector.tensor_tensor(tmp2[:, 1:2], mv[:, 1:2], msq[:], ALU.add)
            bc = pp.tile([128, 512], f32, tag="op", bufs=4, name=f"bc{h}")[:, 0:2]
            nc.tensor.matmul(bc, ones_sb[:], tmp2[:], start=True, stop=True)
            mean = gpool.tile([128, 1], f32, tag="mean", name="mean")
            nc.vector.tensor_scalar_mul(mean[:], bc[:, 0:1], 1.0 / 128)
            e2 = gpool.tile([128, 1], f32, tag="e2", name="e2")
            nc.vector.tensor_scalar_mul(e2[:], bc[:, 1:2], 1.0 / 128)
            m2t = gpool.tile([128, 1], f32, tag="m2t", name="m2t")
            nc.vector.tensor_tensor(m2t[:], mean[:], mean[:], ALU.mult)
            var = gpool.tile([128, 1], f32, tag="var", name="var")
            nc.vector.tensor_tensor(var[:], e2[:], m2t[:], ALU.subtract)
            std = gpool.tile([128, 1], f32, tag="std", name="std")
            nc.scalar.activation(std[:], var[:], FT.Sqrt, bias=eps_sb[:])
            rstd = gpool.tile([128, 1], f32, tag="rstd", name="rstd")
            nc.vector.reciprocal(rstd[:], std[:])
            mrs = gpool.tile([128, 1], f32, tag="mrs", name="mrs")
            nc.vector.scalar_tensor_tensor(mrs[:], mean[:], -1.0, rstd[:],
                                           ALU.mult, ALU.mult)
            rstds[h], mrss[h] = rstd, mrs
        # first t-block scaled at fine granularity so phase 3's first
        # accumulation chain unblocks after ~4 short ops
        for h in range(HPC):
            for c in range(2):
                sl = outT_sb[:, h * 2 + c, 0:128]
                nc.vector.tensor_scalar(sl, sl, rstds[h][:], mrss[h][:],
                                        op0=ALU.mult, op1=ALU.add)
        for q4 in range(T // 512):
            for h in range(HPC):
                for c in range(2):
                    lo = q4 * 512 + (128 if q4 == 0 else 0)
                    sl = outT_sb[:, h * 2 + c, lo:(q4 + 1) * 512]
                    nc.vector.tensor_scalar(sl, sl, rstds[h][:], mrss[h][:],
                                            op0=ALU.mult, op1=ALU.add)

    # ================= phase 3: output projection =================
    if True:
        yr = p["y"].rearrange("(n p) m -> p n m", p=128)
        for tb in range(NT):
            yst = ypool.tile([128, 1, D], bf16, tag="yst")
            for ns in range(D // 512):
                py = pp.tile([128, 512], f32, tag="sc", bufs=4,
                             name=f"py{tb}_{ns}")
                for c in range(NCH):
                    nc.tensor.matmul(py[:],
                                     outT_sb[:, c, tb * 128:(tb + 1) * 128],
                                     wo_sb[:, c, ns * 512:(ns + 1) * 512],
                                     start=(c == 0), stop=(c == NCH - 1))
                if ns % 2 == 0:
                    nc.scalar.copy(yst[:, 0, ns * 512:(ns + 1) * 512], py[:])
                else:
                    nc.vector.tensor_copy(yst[:, 0, ns * 512:(ns + 1) * 512],
                                          py[:])
                if tb == NT - 1 and ns == 1:
                    # tail split: bulk first, small last chunk, few HWDGE
                    # descriptors (625ns each), all on the idle sync queue
                    nc.sync.dma_start(yr[:, tb:tb + 1, 0:1024],
                                      yst[:, :, 0:1024])
                if tb == NT - 1 and ns == 2:
                    nc.sync.dma_start(yr[:, tb:tb + 1, 1024:1536],
                                      yst[:, :, 1024:1536])
                if tb == NT - 1 and ns == 3:
                    nc.sync.dma_start(yr[:, tb:tb + 1, 1536:2048],
                                      yst[:, :, 1536:2048])
            if tb < NT - 1:
                eng = nc.sync if tb % 2 == 0 else nc.scalar
                eng.dma_start(yr[:, tb:tb + 1, :], yst[:])


_prog_cache = {}


def _get_program(lam):
    key = round(float(lam), 9)
    if key in _prog_cache:
        return _prog_cache[key]
    nc = bacc.Bacc("TRN2", target_bir_lowering=False, debug=False)
    p = _declare_params(nc)
    with tile.TileContext(nc) as tc:
        with ExitStack() as ctx:
            _build(ctx, tc, p, lam)
    nc.compile()
    _prog_cache[key] = nc
    return nc


# ----------------------------------------------------------------------------
# host-side input prep
# ----------------------------------------------------------------------------

def _rope_tables():
    inv = 1.0 / (ROPE_BASE ** (np.arange(0, DH, 2, dtype=np.float64) / DH))
    freqs = np.arange(T, dtype=np.float64)[:, None] * inv[None, :]   # [T, 64]
    cos, sin = np.cos(freqs), np.sin(freqs)
    ropec = np.empty((128, T), np.float32)
    ropes = np.empty((128, T), np.float32)
    ropec[0::2, :] = cos.T
    ropec[1::2, :] = cos.T
    ropes[0::2, :] = -sin.T
    ropes[1::2, :] = sin.T
    return ropec, ropes


def _const_inputs():
    ropec, ropes = _rope_tables()
    tri = (np.arange(128)[:, None] <= np.arange(128)[None, :]).astype(np.float32)
    ones = np.ones((128, 128), np.float32)
    ident = np.eye(128, dtype=np.float32)
    b = ml_dtypes.bfloat16
    return dict(ropec=ropec.astype(b), ropes=ropes.astype(b),
                tri=tri.astype(b), ones=ones, ident=ident.astype(b))


def make_in_maps(x, Wq, Wk, Wv, Wo, gn_w):
    b = ml_dtypes.bfloat16
    x2d = np.asarray(x, np.float32).reshape(T, D)
    xT = np.ascontiguousarray(x2d.T).astype(b)
    consts = _const_inputs()
    gw = np.asarray(gn_w, np.float64)
    in_maps = []
    for core in range(NCORES):
        h0 = core * HPC
        kv = h0 // REP
        sl = slice(h0 * 2 * DH, (h0 + HPC) * 2 * DH)
        wo = ((1.0 - LAMBDA_INIT) * gw[sl, None]
              * np.asarray(Wo, np.float64)[sl, :]).astype(b)
        in_maps.append(dict(
            xT=xT,
            wq=np.ascontiguousarray(np.asarray(Wq, np.float32)[:, sl]).astype(b),
            wk=np.ascontiguousarray(
                np.asarray(Wk, np.float32)[:, kv * 2 * DH:(kv + 1) * 2 * DH]).astype(b),
            wv=np.ascontiguousarray(
                np.asarray(Wv, np.float32)[:, kv * 2 * DH:(kv + 1) * 2 * DH]).astype(b),
            wo=wo,
            **consts,
        ))
    return in_maps


def kernel(x, Wq, Wk, Wv, Wo, lambda_q1, lambda_k1, lambda_q2, lambda_k2,
           gn_w, gn_b):
    lam = float(np.exp(np.sum(np.asarray(lambda_q1, np.float64)
                              * np.asarray(lambda_k1, np.float64)))
                - np.exp(np.sum(np.asarray(lambda_q2, np.float64)
                                * np.asarray(lambda_k2, np.float64)))
                + LAMBDA_INIT)
    nc = _get_program(lam)
    in_maps = make_in_maps(x, Wq, Wk, Wv, Wo, gn_w)
    res = run_bass_kernel_spmd(nc, in_maps, list(range(NCORES)))
    y = np.zeros((T, D), np.float64)
    for core in range(NCORES):
        y += res.results[core]["y"].astype(np.float64)
    # gn_b contribution: (1-lambda_init) * gn_b @ Wo, constant over t
    y += (1.0 - LAMBDA_INIT) * (np.asarray(gn_b, np.float64)
                                @ np.asarray(Wo, np.float64))[None, :]
    return y.astype(np.float32).reshape(B, T, D)


# revision 72
# speedup vs baseline: 1.0038x; 1.0038x over previous
"""Trainium2 Bass kernel for nn_AttnLayerV3 (differential attention layer).

Strategy: tensor-parallel over heads — 16 heads across 8 NeuronCores
(2 heads/core, each core needs exactly one of the 4 GQA KV heads).
Each core receives x^T plus its weight slices, computes its heads'
attention + per-head GroupNorm + its slice of the output projection,
and returns a partial [T, D] product; the host sums the 8 partials.

All tensors stream as bf16 (halves DMA + SBUF, 2x DVE rate; PE rate is
1 col/cycle for both bf16 and f32r, accumulation stays fp32 in PSUM).
GroupNorm weight and the (1 - lambda_init) factor are folded into Wo on
the host; the gn_b term is a constant row vector added on the host.
Phase 1 walks t-strips forward so phase 2 (ascending t-strips) never
waits on RoPE; PSUM pools are sized for full cross-strip double
buffering to keep the PE p-state warm.
"""

import math
from contextlib import ExitStack

import numpy as np
import ml_dtypes

import concourse.bacc as bacc
import concourse.tile as tile
from concourse import mybir
from concourse.bass_utils import run_bass_kernel_spmd

f32 = mybir.dt.float32
f32r = mybir.dt.float32r
bf16 = mybir.dt.bfloat16
FT = mybir.ActivationFunctionType
ALU = mybir.AluOpType

# problem shape (hardcoded per contract)
B, T, D, H, KV = 1, 2048, 2048, 16, 4
DH = D // H                    # 128
REP = H // KV                  # 4
NCORES = 8
HPC = H // NCORES              # heads per core = 2
CH = HPC * 2 * DH              # 512 output channels per core
LAMBDA_INIT = 0.8 - 0.6 * math.exp(-0.3 * 0)  # 0.2
ROPE_BASE = 10000.0
EPS = 1e-5

SWAP_MASK = [i ^ 1 for i in range(32)]


# ----------------------------------------------------------------------------
# device program
# ----------------------------------------------------------------------------

def _declare_params(nc):
    p = {}
    p["xT"] = nc.dram_tensor("xT", [D, T], bf16, kind="ExternalInput").ap()
    p["wq"] = nc.dram_tensor("wq", [D, CH], bf16, kind="ExternalInput").ap()
    p["wk"] = nc.dram_tensor("wk", [D, 2 * DH], bf16, kind="ExternalInput").ap()
    p["wv"] = nc.dram_tensor("wv", [D, 2 * DH], bf16, kind="ExternalInput").ap()
    p["wo"] = nc.dram_tensor("wo", [CH, D], bf16, kind="ExternalInput").ap()
    p["ropec"] = nc.dram_tensor("ropec", [128, T], bf16, kind="ExternalInput").ap()
    p["ropes"] = nc.dram_tensor("ropes", [128, T], bf16, kind="ExternalInput").ap()
    p["tri"] = nc.dram_tensor("tri", [128, 128], bf16, kind="ExternalInput").ap()
    p["ones"] = nc.dram_tensor("ones", [128, 128], f32r, kind="ExternalInput").ap()
    p["ident"] = nc.dram_tensor("ident", [128, 128], bf16, kind="ExternalInput").ap()
    p["y"] = nc.dram_tensor("y", [T, D], bf16, kind="ExternalOutput").ap()
    return p


def _build(ctx, tc, p, lam):
    nc = tc.nc
    ND = D // 128          # contraction chunks
    NT = T // 128          # t subtiles
    S1 = 256               # phase-1 t-strip width
    NS1 = T // S1
    CH = HPC * 2 * DH      # output channels this core
    NCH = CH // 128
    NQ = 2 * HPC           # q rows (h-major, [q1,q2] minor)
    scale = 1.0 / math.sqrt(DH)

    # ---------------- constant tiles (resident) ----------------
    consts = ctx.enter_context(tc.tile_pool(name="consts", bufs=1))
    ropec_sb = consts.tile([128, T], bf16, tag="ropec")
    ropes_sb = consts.tile([128, T], bf16, tag="ropes")
    tri_sb = consts.tile([128, 128], bf16, tag="tri")
    ones_sb = consts.tile([128, 128], f32r, tag="ones")
    ident_sb = consts.tile([128, 128], bf16, tag="ident")
    eps_sb = consts.tile([128, 1], f32, tag="eps")
    nc.vector.memset(eps_sb[:], EPS)

    # outT outlives the q/k/v tensors (read by phase 3), so its pool sits
    # below them on the pool stack.
    persist = ctx.enter_context(tc.tile_pool(name="persist", bufs=1))
    outT_sb = persist.tile([128, NCH, T], bf16, tag="outT")      # [ch, chunk, t]

    acts = ctx.enter_context(tc.tile_pool(name="acts", bufs=1))
    qT_sb = acts.tile([128, NQ, T], bf16, tag="qT")              # [dh, (h,b), t]
    kT_sb = acts.tile([128, 2, T], bf16, tag="kT")               # [dh, b, t]
    vaug_sb = acts.tile([128, NT, 2 * DH + 2], bf16, tag="vaug")  # [s, sblock, ch|1]
    nc.vector.memset(vaug_sb[:, :, 2 * DH:2 * DH + 2], 1.0)

    # phase-2/3 SBUF pools created before phase 1's transient pools so their
    # zones never overlap a released zone (no cross-phase barrier waits).
    wop = ctx.enter_context(tc.tile_pool(name="wop", bufs=1))
    wo_sb = wop.tile([128, NCH, D], bf16, tag="wo")
    wor = p["wo"].rearrange("(n p) m -> p n m", p=128)
    epool = ctx.enter_context(tc.tile_pool(name="exp", bufs=24))
    o1pool = ctx.enter_context(tc.tile_pool(name="o1sp", bufs=10))
    cpool = ctx.enter_context(tc.tile_pool(name="comb", bufs=4))
    spool = ctx.enter_context(tc.tile_pool(name="small", bufs=8))
    gpool = ctx.enter_context(tc.tile_pool(name="gn", bufs=2))
    ypool = ctx.enter_context(tc.tile_pool(name="yst", bufs=4))

    # one PSUM pool for all phases: phase-1 accumulators borrow the
    # phase-2/3 tags (even strips: op x4, odd strips: sc x3 + pt x1) so no
    # pool-release barrier separates the phases.
    pp = ctx.enter_context(tc.tile_pool(name="pp", bufs=1, space="PSUM"))

    # ================= phase 1: projections + RoPE =================
    def rope_pass(rpool, dest, ps, t0, w, ceng):
        raw = rpool.tile([128, w], bf16, tag="raw", name="raw")
        if ceng == 0:
            nc.scalar.copy(raw[:], ps)
        else:
            nc.vector.tensor_copy(raw[:], ps)
        swp = rpool.tile([128, w], bf16, tag="swp", name="swp")
        nc.vector.stream_shuffle(swp[:], raw[:], SWAP_MASK)
        nc.gpsimd.tensor_tensor(dest, raw[:], ropec_sb[:, t0:t0 + w], ALU.mult)
        nc.vector.tensor_tensor(swp[:], swp[:], ropes_sb[:, t0:t0 + w], ALU.mult)
        nc.vector.tensor_tensor(dest, dest, swp[:], ALU.add)

    with ExitStack() as ph1:
        wts = ph1.enter_context(tc.tile_pool(name="wts", bufs=1))
        wq_sb = wts.tile([128, ND, CH], bf16, tag="wq")
        wk_sb = wts.tile([128, ND, 2 * DH], bf16, tag="wk")
        wv_sb = wts.tile([128, ND, 2 * DH], bf16, tag="wv")
        wqr = p["wq"].rearrange("(n p) m -> p n m", p=128)
        wkr = p["wk"].rearrange("(n p) m -> p n m", p=128)
        wvr = p["wv"].rearrange("(n p) m -> p n m", p=128)
        for dg in range(ND // 4):
            if dg == 0:
                # d0 alone first so the first matmuls unblock early, then
                # the rest of the group as one descriptor
                nc.scalar.dma_start(wq_sb[:, 0:1, :], wqr[:, 0:1, :])
                nc.scalar.dma_start(wq_sb[:, 1:4, :], wqr[:, 1:4, :])
            else:
                dsl = slice(dg * 4, (dg + 1) * 4)
                nc.scalar.dma_start(wq_sb[:, dsl, :], wqr[:, dsl, :])
            if dg == ND // 4 - 1:
                # first halves only: strip 0's rope needs cols 0:256; the
                # rest loads once the startup bandwidth crunch is over
                nc.scalar.dma_start(ropec_sb[:, 0:1024], p["ropec"][:, 0:1024])
                nc.scalar.dma_start(ropes_sb[:, 0:1024], p["ropes"][:, 0:1024])


        xpool = ph1.enter_context(tc.tile_pool(name="xt", bufs=8))
        rpool = ph1.enter_context(tc.tile_pool(name="rope", bufs=3))

        DB = 4  # d-chunks per x DMA
        xTr = p["xT"].rearrange("(n p) m -> p n m", p=128)
        # strip list: 7 x 256 plus a split 128+128 tail so the final
        # rope-drain chain (which gates phase 2) is half as long
        strips = [(i * S1, S1) for i in range(NS1 - 1)]
        strips += [(T - 2 * 128, 128), (T - 128, 128)]
        for s, (t0, w) in enumerate(strips):
            # acc0 = q j0|j1, acc1 = q j2|j3, acc2 = k b0|b1, acc3 = v s0|s1
            if s % 2 == 0:
                accs = [pp.tile([128, 2 * w], f32, tag="op", bufs=4,
                                name=f"acc{_i}") for _i in range(4)]
            else:
                accs = [pp.tile([128, 2 * w], f32, tag="sc", bufs=4,
                                name=f"acc{_i}") for _i in range(3)]
                accs.append(pp.tile([128, 2 * w], f32, tag="op", bufs=4,
                                    name="acc3"))
            for dg in range(ND // DB):
                xt = xpool.tile([128, DB, S1], bf16, tag="xt",
                                padded_shape=[128, DB, S1])
                nc.sync.dma_start(xt[:, :, 0:w],
                                  xTr[:, dg * DB:(dg + 1) * DB, t0:t0 + w])
                if s == 0 and dg < ND // DB:
                    dsl = slice(dg * DB, (dg + 1) * DB)
                    nc.sync.dma_start(wk_sb[:, dsl, :], wkr[:, dsl, :])
                    nc.sync.dma_start(wv_sb[:, dsl, :], wvr[:, dsl, :])
                if s == 1 and dg == 0:
                    nc.scalar.dma_start(tri_sb[:], p["tri"][:])
                    nc.scalar.dma_start(ones_sb[:], p["ones"][:])
                    nc.scalar.dma_start(ident_sb[:], p["ident"][:])
                if s == 2 and dg == 0:
                    nc.scalar.dma_start(ropec_sb[:, 1024:], p["ropec"][:, 1024:])
                    nc.scalar.dma_start(ropes_sb[:, 1024:], p["ropes"][:, 1024:])
                if s == 4 and dg == 0:
                    for c in range(NCH):
                        nc.scalar.dma_start(wo_sb[:, c, :], wor[:, c, :])
                for dq in range(DB):
                    d = dg * DB + dq
                    # Two accumulation groups share each PSUM bank: start=True
                    # zeroes the whole 2KB zero-region, so only the lo half
                    # starts the bank and only the hi half stops it; the hi
                    # group's first matmul accumulates onto the zeroed region.
                    for j in range(NQ):
                        nc.tensor.matmul(
                            accs[j // 2][:, (j % 2) * w:(j % 2 + 1) * w],
                            wq_sb[:, d, j * 128:(j + 1) * 128],
                            xt[:, dq, 0:w],
                            start=(d == 0 and j % 2 == 0),
                            stop=(d == ND - 1 and j % 2 == 1))
                    for j in range(2):
                        nc.tensor.matmul(
                            accs[2][:, j * w:(j + 1) * w],
                            wk_sb[:, d, j * 128:(j + 1) * 128],
                            xt[:, dq, 0:w],
                            start=(d == 0 and j == 0),
                            stop=(d == ND - 1 and j == 1))
                    for js in range(w // 128):
                        nc.tensor.matmul(
                            accs[3][:, js * 2 * DH:js * 2 * DH + 2 * DH],
                            xt[:, dq, js * 128:(js + 1) * 128],
                            wv_sb[:, d, :],
                            start=(d == 0 and js == 0),
                            stop=(d == ND - 1 and js == w // 128 - 1))
            last = (s >= len(strips) - 2)
            for j in range(2):
                rope_pass(rpool, kT_sb[:, j, t0:t0 + w],
                          accs[2][:, j * w:(j + 1) * w], t0, w,
                          1 if last else j % 2)
            for j in range(NQ):
                rope_pass(rpool, qT_sb[:, j, t0:t0 + w],
                          accs[j // 2][:, (j % 2) * w:(j % 2 + 1) * w], t0, w,
                          1 if last else j % 2)
            for js in range(w // 128):
                nc.scalar.copy(vaug_sb[:, t0 // 128 + js, 0:2 * DH],
                               accs[3][:, js * 2 * DH:js * 2 * DH + 2 * DH])

    # ================= phases 2+3 share one PSUM pool =================

    if True:
        SW = 512                     # phase-2 strip width
        NSW = T // SW
        nkk = SW // 128              # 4 t-subtiles per strip
        statmap = {}
        for h in range(HPC):
            stats = gpool.tile([128, NT, 6], f32, tag="stats", name=f"stats{h}")
            ptile = {}
            for jt in range(NSW):
                t0 = jt * SW
                nsb = (t0 + SW) // 128
                o1sp = {}                     # kk -> o1|r1 spilled to SBUF
                op = {}
                for b in (0, 1):
                    ets = {}
                    # wave A: scores + exp + PV chains for kk in 0..nkk/2
                    for kk in range(nkk // 2):
                        op[kk] = pp.tile([128, 512], f32, tag="op", bufs=4,
                                         name=f"op{b}_{kk}")
                    for sb in range(nsb):
                        sc = pp.tile([128, SW], f32, tag="sc", bufs=4)
                        et = epool.tile([128, SW], bf16, tag="et")
                        if sb * 128 >= t0:   # diagonal sub-block: mask s > t
                            kd = sb - t0 // 128
                            c0 = kd * 128
                            nc.tensor.matmul(sc[:, c0:],
                                             kT_sb[:, b, sb * 128:(sb + 1) * 128],
                                             qT_sb[:, h * 2 + b, t0 + c0:t0 + SW],
                                             start=True, stop=True)
                            nc.scalar.activation(et[:, kd * 128:], sc[:, kd * 128:],
                                                 FT.Exp, scale=scale)
                            nc.gpsimd.tensor_tensor(
                                et[:, kd * 128:(kd + 1) * 128],
                                et[:, kd * 128:(kd + 1) * 128], tri_sb[:], ALU.mult)
                        else:
                            nc.tensor.matmul(sc[:],
                                             kT_sb[:, b, sb * 128:(sb + 1) * 128],
                                             qT_sb[:, h * 2 + b, t0:t0 + SW],
                                             start=True, stop=True)
                            nc.scalar.activation(et[:], sc[:], FT.Exp, scale=scale)
                        ets[sb] = et
                        for kk in range(nkk // 2):
                            tb = t0 // 128 + kk
                            if sb > tb:
                                continue
                            nc.tensor.matmul(op[kk][:, 0:2 * DH + 2],
                                             et[:, kk * 128:(kk + 1) * 128],
                                             vaug_sb[:, sb, :],
                                             start=(sb == 0), stop=(sb == tb))
                    if b == 0:
                        for kk in range(nkk // 2):
                            o1sp[kk] = o1pool.tile([128, 2 * DH + 2], f32,
                                                   tag="o1sp", name=f"o1sp{kk}")
                            nc.vector.tensor_copy(o1sp[kk][:],
                                                  op[kk][:, 0:2 * DH + 2])
                    # wave B: pure-PE PV chains for kk in nkk/2..nkk (re-reads et)
                    for kk in range(nkk // 2, nkk):
                        op[kk] = pp.tile([128, 512], f32, tag="op", bufs=4,
                                         name=f"op{b}_{kk}")
                    for sb in range(nsb):
                        for kk in range(nkk // 2, nkk):
                            tb = t0 // 128 + kk
                            if sb > tb:
                                continue
                            nc.tensor.matmul(op[kk][:, 0:2 * DH + 2],
                                             ets[sb][:, kk * 128:(kk + 1) * 128],
                                             vaug_sb[:, sb, :],
                                             start=(sb == 0), stop=(sb == tb))
                    if b == 0:
                        for kk in range(nkk // 2, nkk):
                            o1sp[kk] = o1pool.tile([128, 2 * DH + 2], f32,
                                                   tag="o1sp", name=f"o1sp{kk}")
                            nc.vector.tensor_copy(o1sp[kk][:],
                                                  op[kk][:, 0:2 * DH + 2])
                def combine(kk):
                    tb = t0 // 128 + kk
                    inv1 = spool.tile([128, 1], f32, tag="inv1")
                    nc.vector.reciprocal(inv1[:], o1sp[kk][:, 2 * DH:2 * DH + 1])
                    inv2 = spool.tile([128, 1], f32, tag="inv2")
                    nc.vector.reciprocal(inv2[:], op[kk][:, 2 * DH:2 * DH + 1])
                    nlinv2 = spool.tile([128, 1], f32, tag="nlinv2")
                    nc.vector.tensor_scalar_mul(nlinv2[:], inv2[:], -lam)
                    o1s = cpool.tile([128, 2 * DH], f32, tag="o1s")
                    nc.vector.tensor_scalar_mul(o1s[:], o1sp[kk][:, 0:2 * DH],
                                                inv1[:])
                    ot = cpool.tile([128, 2 * DH], bf16, tag="ot")
                    nc.vector.scalar_tensor_tensor(ot[:], op[kk][:, 0:2 * DH],
                                                   nlinv2[:], o1s[:],
                                                   ALU.mult, ALU.add)
                    nc.vector.bn_stats(stats[:, tb, :], ot[:])
                    if tb % 2 == 0:
                        ptile[0] = pp.tile([128, 512], bf16, tag="op", bufs=4,
                                           name="pt")
                    for c in range(2):
                        nc.tensor.matmul(
                            ptile[0][:, (c * 2 + tb % 2) * 128:
                                     (c * 2 + tb % 2 + 1) * 128],
                            ot[:, c * 128:(c + 1) * 128], ident_sb[:],
                            is_transpose=True,
                            start=(tb % 2 == 0 and c == 0),
                            stop=(tb % 2 == 1 and c == 1))
                    if tb % 2 == 1:
                        for c in range(2):
                            dst = outT_sb[:, h * 2 + c,
                                          (tb - 1) * 128:(tb + 1) * 128]
                            nc.vector.tensor_copy(
                                dst, ptile[0][:, c * 256:(c + 1) * 256])

                for kk in range(nkk):
                    combine(kk)

            statmap[h] = stats


        # ---- GroupNorm for both heads at phase-2 end: a single act-table
        # switch (Sqrt), and the scale pass runs q4-outer so phase 3's first
        # t-blocks unblock after one chunk per channel tile.
        rstds, mrss = {}, {}
        for h in range(HPC):
            stats = statmap[h]
            mv = gpool.tile([128, 2], f32, tag="mv", name="mv")
            nc.vector.bn_aggr(mv[:], stats[:])
            msq = gpool.tile([128, 1], f32, tag="msq", name="msq")
            nc.vector.tensor_tensor(msq[:], mv[:, 0:1], mv[:, 0:1], ALU.mult)
            tmp2 = gpool.tile([128, 2], f32r, tag="tmp2", name="tmp2")
            nc.vector.tensor_copy(tmp2[:, 0:1], mv[:, 0:1])
            nc.vector.tensor_tensor(tmp2[:, 1:2], mv[:, 1:2], msq[:], ALU.add)
            bc = pp.tile([128, 512], f32, tag="op", bufs=4, name=f"bc{h}")[:, 0:2]
            nc.tensor.matmul(bc, ones_sb[:], tmp2[:], start=True, stop=True)
            mean = gpool.tile([128, 1], f32, tag="mean", name="mean")
            nc.vector.tensor_scalar_mul(mean[:], bc[:, 0:1], 1.0 / 128)
            e2 = gpool.tile([128, 1], f32, tag="e2", name="e2")
            nc.vector.tensor_scalar_mul(e2[:], bc[:, 1:2], 1.0 / 128)
            m2t = gpool.tile([128, 1], f32, tag="m2t", name="m2t")
            nc.vector.tensor_tensor(m2t[:], mean[:], mean[:], ALU.mult)
            var = gpool.tile([128, 1], f32, tag="var", name="var")
            nc.vector.tensor_tensor(var[:], e2[:], m2t[:], ALU.subtract)
            std = gpool.tile([128, 1], f32, tag="std", name="std")
            nc.scalar.activation(std[:], var[:], FT.Sqrt, bias=eps_sb[:])
            rstd = gpool.tile([128, 1], f32, tag="rstd", name="rstd")
            nc.vector.reciprocal(rstd[:], std[:])
            mrs = gpool.tile([128, 1], f32, tag="mrs", name="mrs")
            nc.vector.scalar_tensor_tensor(mrs[:], mean[:], -1.0, rstd[:],
                                           ALU.mult, ALU.mult)
            rstds[h], mrss[h] = rstd, mrs
        # first t-block scaled at fine granularity so phase 3's first
        # accumulation chain unblocks after ~4 short ops
        for h in range(HPC):
            for c in range(2):
                sl = outT_sb[:, h * 2 + c, 0:128]
                nc.vector.tensor_scalar(sl, sl, rstds[h][:], mrss[h][:],
                                        op0=ALU.mult, op1=ALU.add)
        for q4 in range(T // 512):
            for h in range(HPC):
                for c in range(2):
                    lo = q4 * 512 + (128 if q4 == 0 else 0)
                    sl = outT_sb[:, h * 2 + c, lo:(q4 + 1) * 512]
                    nc.vector.tensor_scalar(sl, sl, rstds[h][:], mrss[h][:],
                                            op0=ALU.mult, op1=ALU.add)

    # ================= phase 3: output projection =================
    if True:
        yr = p["y"].rearrange("(n p) m -> p n m", p=128)
        for tb in range(NT):
            yst = ypool.tile([128, 1, D], bf16, tag="yst")
            for ns in range(D // 512):
                py = pp.tile([128, 512], f32, tag="sc", bufs=4,
                             name=f"py{tb}_{ns}")
                for c in range(NCH):
                    nc.tensor.matmul(py[:],
                                     outT_sb[:, c, tb * 128:(tb + 1) * 128],
                                     wo_sb[:, c, ns * 512:(ns + 1) * 512],
                                     start=(c == 0), stop=(c == NCH - 1))
                if ns % 2 == 0:
                    nc.scalar.copy(yst[:, 0, ns * 512:(ns + 1) * 512], py[:])
                else:
                    nc.vector.tensor_copy(yst[:, 0, ns * 512:(ns + 1) * 512],
                                          py[:])
                if tb == NT - 1 and ns == 1:
                    # tail split: bulk first, small last chunk, few HWDGE
                    # descriptors (625ns each), all on the idle sync queue
                    nc.sync.dma_start(yr[:, tb:tb + 1, 0:1024],
                                      yst[:, :, 0:1024])
                if tb == NT - 1 and ns == 2:
                    nc.sync.dma_start(yr[:, tb:tb + 1, 1024:1536],
                                      yst[:, :, 1024:1536])
                if tb == NT - 1 and ns == 3:
                    nc.sync.dma_start(yr[:, tb:tb + 1, 1536:2048],
                                      yst[:, :, 1536:2048])
            if tb < NT - 1:
                eng = nc.sync if tb % 2 == 0 else nc.scalar
                eng.dma_start(yr[:, tb:tb + 1, :], yst[:])


_prog_cache = {}


def _get_program(lam):
    key = round(float(lam), 9)
    if key in _prog_cache:
        return _prog_cache[key]
    nc = bacc.Bacc("TRN2", target_bir_lowering=False, debug=False)
    p = _declare_params(nc)
    with tile.TileContext(nc) as tc:
        with ExitStack() as ctx:
            _build(ctx, tc, p, lam)
    nc.compile()
    _prog_cache[key] = nc
    return nc


# ----------------------------------------------------------------------------
# host-side input prep
# ----------------------------------------------------------------------------

def _rope_tables():
    inv = 1.0 / (ROPE_BASE ** (np.arange(0, DH, 2, dtype=np.float64) / DH))
    freqs = np.arange(T, dtype=np.float64)[:, None] * inv[None, :]   # [T, 64]
    cos, sin = np.cos(freqs), np.sin(freqs)
    ropec = np.empty((128, T), np.float32)
    ropes = np.empty((128, T), np.float32)
    ropec[0::2, :] = cos.T
    ropec[1::2, :] = cos.T
    ropes[0::2, :] = -sin.T
    ropes[1::2, :] = sin.T
    return ropec, ropes


def _const_inputs():
    ropec, ropes = _rope_tables()
    tri = (np.arange(128)[:, None] <= np.arange(128)[None, :]).astype(np.float32)
    ones = np.ones((128, 128), np.float32)
    ident = np.eye(128, dtype=np.float32)
    b = ml_dtypes.bfloat16
    return dict(ropec=ropec.astype(b), ropes=ropes.astype(b),
                tri=tri.astype(b), ones=ones, ident=ident.astype(b))


def make_in_maps(x, Wq, Wk, Wv, Wo, gn_w):
    b = ml_dtypes.bfloat16
    x2d = np.asarray(x, np.float32).reshape(T, D)
    xT = np.ascontiguousarray(x2d.T).astype(b)
    consts = _const_inputs()
    gw = np.asarray(gn_w, np.float64)
    in_maps = []
    for core in range(NCORES):
        h0 = core * HPC
        kv = h0 // REP
        sl = slice(h0 * 2 * DH, (h0 + HPC) * 2 * DH)
        wo = ((1.0 - LAMBDA_INIT) * gw[sl, None]
              * np.asarray(Wo, np.float64)[sl, :]).astype(b)
        in_maps.append(dict(
            xT=xT,
            wq=np.ascontiguousarray(np.asarray(Wq, np.float32)[:, sl]).astype(b),
            wk=np.ascontiguousarray(
                np.asarray(Wk, np.float32)[:, kv * 2 * DH:(kv + 1) * 2 * DH]).astype(b),
            wv=np.ascontiguousarray(
                np.asarray(Wv, np.float32)[:, kv * 2 * DH:(kv + 1) * 2 * DH]).astype(b),
            wo=wo,
            **consts,
        ))
    return in_maps


def kernel(x, Wq, Wk, Wv, Wo, lambda_q1, lambda_k1, lambda_q2, lambda_k2,
           gn_w, gn_b):
    lam = float(np.exp(np.sum(np.asarray(lambda_q1, np.float64)
                              * np.asarray(lambda_k1, np.float64)))
                - np.exp(np.sum(np.asarray(lambda_q2, np.float64)
                                * np.asarray(lambda_k2, np.float64)))
                + LAMBDA_INIT)
    nc = _get_program(lam)
    in_maps = make_in_maps(x, Wq, Wk, Wv, Wo, gn_w)
    res = run_bass_kernel_spmd(nc, in_maps, list(range(NCORES)))
    y = np.zeros((T, D), np.float64)
    for core in range(NCORES):
        y += res.results[core]["y"].astype(np.float64)
    # gn_b contribution: (1-lambda_init) * gn_b @ Wo, constant over t
    y += (1.0 - LAMBDA_INIT) * (np.asarray(gn_b, np.float64)
                                @ np.asarray(Wo, np.float64))[None, :]
    return y.astype(np.float32).reshape(B, T, D)
